# revision 8
# baseline (speedup 1.0000x reference)
"""ConvLSTM2D forward on 8 Trainium2 NeuronCores (v4).

Problem: x [8,10,256,256,8], Wx [3,3,8,4], Wh [3,3,1,4], b [4]
         -> h_last [8,256,256,1]  (ConvLSTM, keras gate order i,f,c,o;
         i/f/o hard_sigmoid, candidate+output sigmoid)

Sharding: data-parallel over batch; core b computes batch element b fully
locally (recurrent scan stays on-core, no collectives in forward).

v4 (v3 was 213us): trace showed PE warm-capable (218ns/MM) but starving:
 - x loads were 4 strided DMAs/step (~2KB packets, ~122GB/s aggregate) ->
   PE idle 68us total, HAM re-throttled to 1.2GHz ~50% of the span.
   Fix: ONE contiguous 1.26MB dma_start per step (per-partition 12384B
   lines, split across all 16 SDMA engines -> ~340GB/s).
 - h scatter was 10 SBUF->SBUF DMAs on the recurrent critical path.
   Fix: h = min(relu_o,1)*sigmoid(c) is computed by scalar_tensor_tensor
   DVE/GpSimd ops writing STRAIGHT into the next x tile's halo windows.
 - 9 weight DMAs serialized ~9us of startup -> single contiguous DMA on
   the scalar queue, concurrent with the x0 load on sync.
 - epilogue split per tau-pair across ACT/DVE/GpSimd; output DMA per pair.
"""

import numpy as np
import ml_dtypes

import concourse.bacc as bacc
import concourse.bass as bass
import concourse.mybir as mybir
import concourse.tile as tile
from concourse import bass_utils

F32 = mybir.dt.float32
BF16 = mybir.dt.bfloat16
AF = mybir.ActivationFunctionType
OP = mybir.AluOpType

B, T, H, W, CIN = 8, 10, 256, 256, 8
G = 4            # gates i,f,c,o
RT = 32          # output rows per tile (M = G*RT = 128)
TAU = H // RT    # 8 row tiles
HIN = RT + 2     # input rows per tile (with halo)
CPG = 3          # channels per contraction group
NCG = 3          # channel groups (3,3,2+h)
KP = HIN * CPG   # 102 partitions per rhs tile
NPAIR = TAU // 2 # 4 tau-pairs (tau, tau+4) -> N=512 matmuls
WP = W + 2       # padded width


def h_window_segments():
    """(tau, seg_lo, seg_hi, planar_part0, planar_blk) for the h halo windows.

    Window rows for tau: 32*tau-1 .. 32*tau+32 (lr 0..33) at partition 68+lr;
    segments split where the window crosses the planar block boundary.
    """
    out = []
    for tau in range(TAU):
        r0 = tau * RT - 1
        lo = max(0, -r0)
        hi = min(HIN, H - r0)
        s = lo
        while s < hi:
            blk = (r0 + s) // 128
            e = min(hi, (blk + 1) * 128 - r0)
            out.append((tau, s, e, r0 + s - blk * 128, blk))
            s = e
    return out


def _seg_max_pair(seg):
    """Highest tau-pair index whose epilogue this window segment needs."""
    tau, s, e, p0, blk = seg
    return max(((p0 + i) % 128) // RT for i in range(e - s))


def pack_inputs(x, Wx, Wh):
    """Host-side repack to bf16 device layouts.

    xk[b, t, cc*34+lr, cg, tau, 1+c] = x[b, t, 32*tau-1+lr, c, 3*cg+cc]
    wb[cc*34+r+kh, 3*cg+kw, g*32+r]  = W9[kh, kw, 3*cg+cc, g]
    """
    x = np.asarray(x, dtype=np.float32)
    W9 = np.concatenate([np.asarray(Wx, np.float32),
                         np.asarray(Wh, np.float32)], axis=2)  # [3,3,9,4]

    xk = np.zeros((B, T, KP, NCG, TAU, WP), dtype=ml_dtypes.bfloat16)
    xb = x.astype(ml_dtypes.bfloat16)
    for tau in range(TAU):
        r0 = tau * RT - 1
        lo = max(0, -r0)
        hi = min(HIN, H - r0)
        for cg in range(NCG):
            for cc in range(CPG):
                ch = cg * CPG + cc
                if ch >= CIN:
                    continue  # h channel: written on device
                xk[:, :, cc * HIN + lo:cc * HIN + hi, cg, tau, 1:W + 1] = \
                    xb[:, :, r0 + lo:r0 + hi, :, ch]

    wb = np.zeros((KP, NCG * 3, G * RT), dtype=np.float32)
    r = np.arange(RT)
    for cg in range(NCG):
        for cc in range(CPG):
            ch = cg * CPG + cc
            for kh in range(3):
                for kw in range(3):
                    for g in range(G):
                        wb[cc * HIN + r + kh, 3 * cg + kw, g * RT + r] = \
                            W9[kh, kw, ch, g]
    return xk, wb.astype(ml_dtypes.bfloat16)


def build_program(Tn, bvals):
    hs_bias = tuple(0.2 * float(v) + 0.5 for v in bvals)  # hard-sigmoid biases
    c_bias = float(bvals[2])
    nc = bacc.Bacc("TRN2", target_bir_lowering=False, debug=False)
    # register const APs for non-Copy activation biases (init covers 0.0/1.0)
    for v in {hs_bias[3], c_bias} - {0.0, 1.0}:
        t = nc.alloc_sbuf_tensor(f"const-f32-{v}", [128, 1], F32)
        nc.gpsimd.memset(t.ap(), v)
        nc.const_aps.aps[(F32, v)] = t.ap()
    if {hs_bias[3], c_bias} - {0.0, 1.0}:
        nc.all_engine_barrier()
    xk_d = nc.dram_tensor("xk", [Tn, KP, NCG, TAU, WP], BF16, kind="ExternalInput")
    wb_d = nc.dram_tensor("wb", [KP, NCG * 3, G * RT], BF16, kind="ExternalInput")
    out_d = nc.dram_tensor("out", [H, W], F32, kind="ExternalOutput")

    segs = h_window_segments()
    # windows grouped by the last pair-epilogue they depend on
    segs_by_pair = {p: [s for s in segs if _seg_max_pair(s) == p]
                    for p in range(NPAIR)}

    with tile.TileContext(nc) as tc:
        with tc.tile_pool(name="wpool", bufs=1) as wpool, \
             tc.tile_pool(name="xpool", bufs=3) as xpool, \
             tc.tile_pool(name="gpool", bufs=2) as gpool, \
             tc.tile_pool(name="state", bufs=1) as state, \
             tc.tile_pool(name="zpsum", bufs=2, space="PSUM") as zpsum:

            # --- static weights / state ---
            wt = wpool.tile([KP, NCG * 3, G * RT], BF16, tag="wt", name="wt")
            nc.scalar.dma_start(out=wt, in_=wb_d[:])

            cbuf = state.tile([128, 2, W], F32, tag="cbuf", name="cbuf")
            nc.vector.memset(cbuf, 0.0)
            hbuf = state.tile([128, 2, WP], BF16, tag="hbuf", name="hbuf")
            nc.vector.memset(hbuf, 0.0)

            def load_x(t):
                xt = xpool.tile([KP, NCG, TAU, WP], BF16, tag="xt", name="xt")
                nc.sync.dma_start(out=xt, in_=xk_d[t])
                return xt

            xt_cur = load_x(0)
            xt_nxt = load_x(1) if Tn > 1 else None
            for t in range(Tn):
                # --- matmuls: 9 accumulating passes x 4 tau-pairs, N=512 ---
                xv = xt_cur.rearrange("p cg (b q) c -> p cg b q c", b=2)
                zt = [zpsum.tile([G * RT, 2, W], F32, tag=f"z{q}", name=f"z{q}")
                      for q in range(NPAIR)]
                gi = gpool.tile([128, 2, W], F32, tag="gi", name="gi")
                gf = gpool.tile([128, 2, W], F32, tag="gf", name="gf")
                go = gpool.tile([128, 2, W], F32, tag="go", name="go")
                sc = gpool.tile([128, 2, W], F32, tag="sc", name="sc")
                s2 = gpool.tile([128, 2, W], F32, tag="s2", name="s2")

                for cg in range(2):
                    for kw in range(3):
                        for q in range(NPAIR):
                            nc.tensor.matmul(
                                zt[q], wt[:, 3 * cg + kw],
                                xv[:, cg, :, q, kw:kw + W],
                                start=(cg == 0 and kw == 0), stop=False)

                def emit_windows(xdst, plist, base):
                    # engine ops can only shift partitions by multiples of
                    # 32; the halo windows need arbitrary shifts -> DMA.
                    engs = (nc.sync, nc.scalar, nc.gpsimd)
                    for n, (tau, s, e, p0, blk) in enumerate(plist):
                        engs[(base + n) % 3].dma_start(
                            out=xdst[68 + s:68 + e, 2, tau, :],
                            in_=hbuf[p0:p0 + (e - s), blk, :])

                for q in range(NPAIR):
                    for kw in range(3):
                        nc.tensor.matmul(
                            zt[q], wt[:, 6 + kw],
                            xv[:, 2, :, q, kw:kw + W],
                            start=False, stop=(kw == 2))
                    # per-pair epilogue; PSUM partitions g*32+r -> planar q*32+r
                    sl = slice(q * RT, (q + 1) * RT)
                    zi, zf, zc, zo = (zt[q][g_ * RT:(g_ + 1) * RT]
                                      for g_ in range(4))
                    # gate affines read PSUM: only DVE/ACT may touch PSUM
                    nc.vector.tensor_scalar(
                        out=gi[sl], in0=zi, scalar1=0.2, scalar2=hs_bias[0],
                        op0=OP.mult, op1=OP.add)
                    nc.vector.tensor_scalar(
                        out=gi[sl], in0=gi[sl], scalar1=0.0, scalar2=1.0,
                        op0=OP.max, op1=OP.min)
                    nc.vector.tensor_scalar(
                        out=gf[sl], in0=zf, scalar1=0.2, scalar2=hs_bias[1],
                        op0=OP.mult, op1=OP.add)
                    nc.gpsimd.tensor_scalar(
                        out=gf[sl], in0=gf[sl], scalar1=0.0, scalar2=1.0,
                        op0=OP.max, op1=OP.min)
                    # o gate: Relu(0.2 z + b); upper clamp fused into windows
                    nc.scalar.activation(
                        out=go[sl], in_=zo, func=AF.Relu,
                        bias=hs_bias[3], scale=0.2)
                    # candidate sigmoid
                    nc.scalar.activation(
                        out=sc[sl], in_=zc, func=AF.Sigmoid,
                        bias=c_bias, scale=1.0)
                    # cell update: c = min(gf,1)*c + gi*sc
                    nc.gpsimd.tensor_tensor(
                        out=gf[sl], in0=gf[sl], in1=cbuf[sl], op=OP.mult)
                    nc.vector.tensor_tensor(
                        out=gi[sl], in0=gi[sl], in1=sc[sl], op=OP.mult)
                    nc.gpsimd.tensor_tensor(
                        out=cbuf[sl], in0=gi[sl], in1=gf[sl], op=OP.add)
                    nc.scalar.activation(
                        out=s2[sl], in_=cbuf[sl], func=AF.Sigmoid,
                        bias=0.0, scale=1.0)

                    # h for this pair; windows whose rows are now complete
                    if xt_nxt is not None:
                        nc.vector.scalar_tensor_tensor(
                            out=hbuf[sl, :, 1:W + 1], in0=go[sl], scalar=1.0,
                            in1=s2[sl], op0=OP.min, op1=OP.mult)
                        if q >= 1:
                            emit_windows(xt_nxt, segs_by_pair[q], q)
                            if q == 1:
                                emit_windows(xt_nxt, segs_by_pair[0], 0)

                if xt_nxt is not None:
                    xt_cur = xt_nxt
                    xt_nxt = load_x(t + 2) if t + 2 < Tn else None
                else:
                    # final step: h = min(go,1)*s2 per pair -> DRAM
                    hf = gpool.tile([128, 2, W], F32, tag="hf", name="hf")
                    ov = out_d.rearrange("(b p) w -> p b w", p=128)
                    oeng = (nc.sync, nc.scalar)
                    for q in range(NPAIR):
                        sl = slice(q * RT, (q + 1) * RT)
                        nc.vector.scalar_tensor_tensor(
                            out=hf[sl], in0=go[sl], scalar=1.0, in1=s2[sl],
                            op0=OP.min, op1=OP.mult)
                        oeng[q % 2].dma_start(out=ov[sl], in_=hf[sl])
    nc.compile()
    return nc


_CACHE = {}


def _get_program(Tn, bvals):
    key = (Tn, bvals)
    if key not in _CACHE:
        _CACHE[key] = build_program(Tn, bvals)
    return _CACHE[key]


def kernel(x, Wx, Wh, b, _run_opts=None):
    x = np.asarray(x, dtype=np.float32)
    b = np.asarray(b, dtype=np.float32)
    Bn, Tn = x.shape[0], x.shape[1]
    xk, wb = pack_inputs(x, Wx, Wh)
    nc = _get_program(Tn, tuple(float(v) for v in b))
    in_maps = [{"xk": np.ascontiguousarray(xk[bi]), "wb": wb}
               for bi in range(Bn)]
    res = bass_utils.run_bass_kernel_spmd(
        nc, in_maps, core_ids=list(range(Bn)), **(_run_opts or {}))
    out = np.stack([res.results[bi]["out"] for bi in range(Bn)], axis=0)
    kernel.last_results = res
    return out[..., None].astype(np.float32)


# revision 12
# speedup vs baseline: 1.8666x; 1.8666x over previous
"""ConvLSTM2D forward on 8 Trainium2 NeuronCores (v4).

Problem: x [8,10,256,256,8], Wx [3,3,8,4], Wh [3,3,1,4], b [4]
         -> h_last [8,256,256,1]  (ConvLSTM, keras gate order i,f,c,o;
         i/f/o hard_sigmoid, candidate+output sigmoid)

Sharding: data-parallel over batch; core b computes batch element b fully
locally (recurrent scan stays on-core, no collectives in forward).

v4 (v3 was 213us): trace showed PE warm-capable (218ns/MM) but starving:
 - x loads were 4 strided DMAs/step (~2KB packets, ~122GB/s aggregate) ->
   PE idle 68us total, HAM re-throttled to 1.2GHz ~50% of the span.
   Fix: ONE contiguous 1.26MB dma_start per step (per-partition 12384B
   lines, split across all 16 SDMA engines -> ~340GB/s).
 - h scatter was 10 SBUF->SBUF DMAs on the recurrent critical path.
   Fix: h = min(relu_o,1)*sigmoid(c) is computed by scalar_tensor_tensor
   DVE/GpSimd ops writing STRAIGHT into the next x tile's halo windows.
 - 9 weight DMAs serialized ~9us of startup -> single contiguous DMA on
   the scalar queue, concurrent with the x0 load on sync.
 - epilogue split per tau-pair across ACT/DVE/GpSimd; output DMA per pair.
"""

import numpy as np
import ml_dtypes

import concourse.bacc as bacc
import concourse.bass as bass
import concourse.mybir as mybir
import concourse.tile as tile
from concourse import bass_utils

F32 = mybir.dt.float32
BF16 = mybir.dt.bfloat16
AF = mybir.ActivationFunctionType
OP = mybir.AluOpType

B, T, H, W, CIN = 8, 10, 256, 256, 8
G = 4            # gates i,f,c,o
RT = 32          # output rows per tile (M = G*RT = 128)
TAU = H // RT    # 8 row tiles
HIN = RT + 2     # input rows per tile (with halo)
CPG = 3          # channels per contraction group
NCG = 3          # channel groups (3,3,2+h)
KP = HIN * CPG   # 102 partitions per rhs tile
NPAIR = TAU // 2 # 4 tau-pairs (tau, tau+4) -> N=512 matmuls
WP = W + 2       # padded width


def h_window_segments():
    """(tau, seg_lo, seg_hi, planar_part0, planar_blk) for the h halo windows.

    Window rows for tau: 32*tau-1 .. 32*tau+32 (lr 0..33) at partition 68+lr;
    segments split where the window crosses the planar block boundary.
    """
    out = []
    for tau in range(TAU):
        r0 = tau * RT - 1
        lo = max(0, -r0)
        hi = min(HIN, H - r0)
        s = lo
        while s < hi:
            blk = (r0 + s) // 128
            e = min(hi, (blk + 1) * 128 - r0)
            out.append((tau, s, e, r0 + s - blk * 128, blk))
            s = e
    return out


def _seg_max_pair(seg):
    """Highest tau-pair index whose epilogue this window segment needs."""
    tau, s, e, p0, blk = seg
    return max(((p0 + i) % 128) // RT for i in range(e - s))


def pack_inputs(x, Wx, Wh):
    """Host-side repack to bf16 device layouts.

    xk[b, t, cc*34+lr, cg, tau, 1+c] = x[b, t, 32*tau-1+lr, c, 3*cg+cc]
    wb[cc*34+r+kh, 3*cg+kw, g*32+r]  = W9[kh, kw, 3*cg+cc, g]
    """
    x = np.asarray(x, dtype=np.float32)
    W9 = np.concatenate([np.asarray(Wx, np.float32),
                         np.asarray(Wh, np.float32)], axis=2)  # [3,3,9,4]

    xk = np.zeros((B, T, KP, NCG, TAU, WP), dtype=ml_dtypes.bfloat16)
    xb = x.astype(ml_dtypes.bfloat16)
    for tau in range(TAU):
        r0 = tau * RT - 1
        lo = max(0, -r0)
        hi = min(HIN, H - r0)
        for cg in range(NCG):
            for cc in range(CPG):
                ch = cg * CPG + cc
                if ch >= CIN:
                    continue  # h channel: written on device
                xk[:, :, cc * HIN + lo:cc * HIN + hi, cg, tau, 1:W + 1] = \
                    xb[:, :, r0 + lo:r0 + hi, :, ch]

    wb = np.zeros((KP, NCG * 3, G * RT), dtype=np.float32)
    r = np.arange(RT)
    for cg in range(NCG):
        for cc in range(CPG):
            ch = cg * CPG + cc
            for kh in range(3):
                for kw in range(3):
                    for g in range(G):
                        wb[cc * HIN + r + kh, 3 * cg + kw, g * RT + r] = \
                            W9[kh, kw, ch, g]
    return xk, wb.astype(ml_dtypes.bfloat16)


def build_program(Tn, bvals):
    hs_bias = tuple(0.2 * float(v) + 0.5 for v in bvals)  # hard-sigmoid biases
    c_bias = float(bvals[2])
    nc = bacc.Bacc("TRN2", target_bir_lowering=False, debug=False)
    # register const APs for non-Copy activation biases (init covers 0.0/1.0)
    for v in {hs_bias[3], c_bias} - {0.0, 1.0}:
        t = nc.alloc_sbuf_tensor(f"const-f32-{v}", [128, 1], F32)
        nc.gpsimd.memset(t.ap(), v)
        nc.const_aps.aps[(F32, v)] = t.ap()
    if {hs_bias[3], c_bias} - {0.0, 1.0}:
        nc.all_engine_barrier()
    xk_d = nc.dram_tensor("xk", [Tn, KP, NCG, TAU, WP], BF16, kind="ExternalInput")
    wb_d = nc.dram_tensor("wb", [KP, NCG * 3, G * RT], BF16, kind="ExternalInput")
    out_d = nc.dram_tensor("out", [H, W], F32, kind="ExternalOutput")

    segs = h_window_segments()
    # windows grouped by the last pair-epilogue they depend on
    segs_by_pair = {p: [s for s in segs if _seg_max_pair(s) == p]
                    for p in range(NPAIR)}

    with tile.TileContext(nc) as tc:
        with tc.tile_pool(name="wpool", bufs=1) as wpool, \
             tc.tile_pool(name="xpool", bufs=3) as xpool, \
             tc.tile_pool(name="gpool", bufs=2) as gpool, \
             tc.tile_pool(name="state", bufs=1) as state, \
             tc.tile_pool(name="zpsum", bufs=2, space="PSUM") as zpsum:

            # --- static weights / state ---
            wt = wpool.tile([KP, NCG * 3, G * RT], BF16, tag="wt", name="wt")
            nc.scalar.dma_start(out=wt, in_=wb_d[:])

            cbuf = state.tile([128, 2, W], F32, tag="cbuf", name="cbuf")
            nc.vector.memset(cbuf, 0.0)
            hbuf = state.tile([128, 2, WP], BF16, tag="hbuf", name="hbuf")
            nc.vector.memset(hbuf, 0.0)

            def load_x(t):
                xt = xpool.tile([KP, NCG, TAU, WP], BF16, tag="xt", name="xt")
                nc.sync.dma_start(out=xt, in_=xk_d[t])
                return xt

            xt_cur = load_x(0)
            xt_nxt = load_x(1) if Tn > 1 else None
            for t in range(Tn):
                # --- matmuls: 9 accumulating passes x 4 tau-pairs, N=512 ---
                xv = xt_cur.rearrange("p cg (b q) c -> p cg b q c", b=2)
                zt = [zpsum.tile([G * RT, 2, W], F32, tag=f"z{q}", name=f"z{q}")
                      for q in range(NPAIR)]
                gi = gpool.tile([128, 2, W], BF16, tag="gi", name="gi")
                gf = gpool.tile([128, 2, W], BF16, tag="gf", name="gf")
                go = gpool.tile([128, 2, W], BF16, tag="go", name="go")
                sc = gpool.tile([128, 2, W], BF16, tag="sc", name="sc")
                s2 = gpool.tile([128, 2, W], BF16, tag="s2", name="s2")
                t2 = gpool.tile([128, 2, W], F32, tag="t2", name="t2")

                for cg in range(2):
                    for kw in range(3):
                        for q in range(NPAIR):
                            nc.tensor.matmul(
                                zt[q], wt[:, 3 * cg + kw],
                                xv[:, cg, :, q, kw:kw + W],
                                start=(cg == 0 and kw == 0), stop=False)

                def emit_windows(xdst, plist, base):
                    # engine ops can only shift partitions by multiples of
                    # 32; the halo windows need arbitrary shifts -> DMA.
                    engs = (nc.sync, nc.gpsimd)
                    for n, (tau, s, e, p0, blk) in enumerate(plist):
                        engs[(base + n) % 2].dma_start(
                            out=xdst[68 + s:68 + e, 2, tau, :],
                            in_=hbuf[p0:p0 + (e - s), blk, :])

                for q in range(NPAIR):
                    for kw in range(3):
                        nc.tensor.matmul(
                            zt[q], wt[:, 6 + kw],
                            xv[:, 2, :, q, kw:kw + W],
                            start=False, stop=(kw == 2))
                    # per-pair PSUM deinterleave (32p forced by PSUM gate
                    # groups); partitions g*32+r -> planar q*32+r, bf16 out
                    sl = slice(q * RT, (q + 1) * RT)
                    zi, zf, zc, zo = (zt[q][g_ * RT:(g_ + 1) * RT]
                                      for g_ in range(4))
                    nc.vector.tensor_scalar(
                        out=gi[sl], in0=zi, scalar1=0.2, scalar2=hs_bias[0],
                        op0=OP.mult, op1=OP.add)
                    nc.vector.tensor_scalar(
                        out=gf[sl], in0=zf, scalar1=0.2, scalar2=hs_bias[1],
                        op0=OP.mult, op1=OP.add)
                    # o gate: Relu(0.2 z + b); min(.,1) applied full-width
                    nc.scalar.activation(
                        out=go[sl], in_=zo, func=AF.Relu,
                        bias=hs_bias[3], scale=0.2)
                    nc.scalar.activation(
                        out=sc[sl], in_=zc, func=AF.Sigmoid,
                        bias=c_bias, scale=1.0)

                # --- full-width (128p) gate math, bf16 ---
                nc.vector.tensor_scalar(out=gi, in0=gi, scalar1=0.0,
                                        scalar2=1.0, op0=OP.max, op1=OP.min)
                nc.gpsimd.tensor_scalar(out=gf, in0=gf, scalar1=0.0,
                                        scalar2=1.0, op0=OP.max, op1=OP.min)
                nc.gpsimd.tensor_scalar(out=go, in0=go, scalar1=1.0,
                                        scalar2=0.0, op0=OP.min, op1=OP.max)
                nc.gpsimd.tensor_tensor(out=t2, in0=gf, in1=cbuf, op=OP.mult)
                nc.vector.tensor_tensor(out=gi, in0=gi, in1=sc, op=OP.mult)
                nc.vector.tensor_tensor(out=cbuf, in0=gi, in1=t2, op=OP.add)
                nc.scalar.activation(out=s2, in_=cbuf, func=AF.Sigmoid,
                                     bias=0.0, scale=1.0)

                if xt_nxt is not None:
                    nc.vector.tensor_tensor(out=hbuf[:, :, 1:W + 1], in0=go,
                                            in1=s2, op=OP.mult)
                    emit_windows(
                        xt_nxt,
                        sorted(segs, key=lambda g: (g[0] % NPAIR, g[0])), 0)
                    xt_cur = xt_nxt
                    xt_nxt = load_x(t + 2) if t + 2 < Tn else None
                else:
                    # final step: h = go*s2 per pair -> DRAM
                    hf = gpool.tile([128, 2, W], F32, tag="hf", name="hf")
                    ov = out_d.rearrange("(b p) w -> p b w", p=128)
                    oeng = (nc.sync, nc.gpsimd)
                    for q in range(NPAIR):
                        sl = slice(q * RT, (q + 1) * RT)
                        nc.vector.tensor_tensor(
                            out=hf[sl], in0=go[sl], in1=s2[sl], op=OP.mult)
                        oeng[q % 2].dma_start(out=ov[sl], in_=hf[sl])
    nc.compile()
    return nc


_CACHE = {}


def _get_program(Tn, bvals):
    key = (Tn, bvals)
    if key not in _CACHE:
        _CACHE[key] = build_program(Tn, bvals)
    return _CACHE[key]


def kernel(x, Wx, Wh, b, _run_opts=None):
    x = np.asarray(x, dtype=np.float32)
    b = np.asarray(b, dtype=np.float32)
    Bn, Tn = x.shape[0], x.shape[1]
    xk, wb = pack_inputs(x, Wx, Wh)
    nc = _get_program(Tn, tuple(float(v) for v in b))
    in_maps = [{"xk": np.ascontiguousarray(xk[bi]), "wb": wb}
               for bi in range(Bn)]
    res = bass_utils.run_bass_kernel_spmd(
        nc, in_maps, core_ids=list(range(Bn)), **(_run_opts or {}))
    out = np.stack([res.results[bi]["out"] for bi in range(Bn)], axis=0)
    kernel.last_results = res
    return out[..., None].astype(np.float32)


# revision 14
# speedup vs baseline: 2.2460x; 1.2033x over previous
"""ConvLSTM2D forward on 8 Trainium2 NeuronCores (v4.2).

Problem: x [8,10,256,256,8], Wx [3,3,8,4], Wh [3,3,1,4], b [4]
         -> h_last [8,256,256,1]  (ConvLSTM, keras gate order i,f,c,o;
         i/f/o hard_sigmoid, candidate+output sigmoid)

Sharding: data-parallel over batch; core b computes batch element b fully
locally (recurrent scan stays on-core, no collectives in forward).

v4.2 (baseline v3 was 213us):
 - ONE contiguous 1.26MB x-load per step on the gpsimd SWDGE queue (the
   341GB/s path; the sync HWDGE ring only reaches ~126GB/s on 6 engines).
 - gate affine 0.2*z+b+0.5 folded INTO the matmul: weights for i/f/o are
   pre-scaled by 0.2 host-side and K grows to 103 with a constant-1 rhs
   row carrying the biases (added once via the cg0/kw0 pass).  The PSUM
   deinterleave then needs only clamp (DVE) / Relu / Sigmoid (ACT) with
   no activation-bias const APs.
 - all-bf16 epilogue (cell state incl.), full-width ops after the 32p
   deinterleave; GpSimd issues DMAs only (its ALU is ~6x slower than DVE).
 - h scatter into the next x tile's halo windows via 10 DMAs on
   sync/gpsimd; per-pair output DMAs on the final step.
"""

import numpy as np
import ml_dtypes

import concourse.bacc as bacc
import concourse.bass as bass
import concourse.mybir as mybir
import concourse.tile as tile
from concourse import bass_utils

F32 = mybir.dt.float32
BF16 = mybir.dt.bfloat16
AF = mybir.ActivationFunctionType
OP = mybir.AluOpType

B, T, H, W, CIN = 8, 10, 256, 256, 8
G = 4            # gates i,f,c,o
RT = 32          # output rows per tile (M = G*RT = 128)
TAU = H // RT    # 8 row tiles
HIN = RT + 2     # input rows per tile (with halo)
CPG = 3          # channels per contraction group
NCG = 3          # channel groups (3,3,2+h)
KP = HIN * CPG   # 102 partitions of conv data per rhs tile
KB = KP + 1      # +1 constant-1 bias row
NPAIR = TAU // 2 # 4 tau-pairs (tau, tau+4) -> N=512 matmuls
WP = W + 2       # padded width


def h_window_segments():
    """(tau, seg_lo, seg_hi, planar_part0, planar_blk) for the h halo windows.

    Window rows for tau: 32*tau-1 .. 32*tau+32 (lr 0..33) at partition 68+lr;
    segments split where the window crosses the planar block boundary.
    """
    out = []
    for tau in range(TAU):
        r0 = tau * RT - 1
        lo = max(0, -r0)
        hi = min(HIN, H - r0)
        s = lo
        while s < hi:
            blk = (r0 + s) // 128
            e = min(hi, (blk + 1) * 128 - r0)
            out.append((tau, s, e, r0 + s - blk * 128, blk))
            s = e
    return out


def pack_inputs(x, Wx, Wh, b):
    """Host-side repack to bf16 device layouts.

    xk[b, t, cc*34+lr, cg, tau, 1+c] = x[b, t, 32*tau-1+lr, c, 3*cg+cc]
    wb[cc*34+r+kh, 3*cg+kw, g*32+r]  = W9[kh, kw, 3*cg+cc, g] * (0.2 unless
    g==2); wb[102, 0, g*32+r] = hard-sigmoid/raw bias for gate g.
    """
    x = np.asarray(x, dtype=np.float32)
    b = np.asarray(b, dtype=np.float32)
    W9 = np.concatenate([np.asarray(Wx, np.float32),
                         np.asarray(Wh, np.float32)], axis=2)  # [3,3,9,4]
    gscale = np.array([0.2, 0.2, 1.0, 0.2], np.float32)
    gbias = np.array([0.2 * b[0] + 0.5, 0.2 * b[1] + 0.5,
                      b[2], 0.2 * b[3] + 0.5], np.float32)

    xk = np.zeros((B, T, KP, NCG, TAU, WP), dtype=ml_dtypes.bfloat16)
    xb = x.astype(ml_dtypes.bfloat16)
    for tau in range(TAU):
        r0 = tau * RT - 1
        lo = max(0, -r0)
        hi = min(HIN, H - r0)
        for cg in range(NCG):
            for cc in range(CPG):
                ch = cg * CPG + cc
                if ch >= CIN:
                    continue  # h channel: written on device
                xk[:, :, cc * HIN + lo:cc * HIN + hi, cg, tau, 1:W + 1] = \
                    xb[:, :, r0 + lo:r0 + hi, :, ch]

    wb = np.zeros((KB, NCG * 3, G * RT), dtype=np.float32)
    r = np.arange(RT)
    for cg in range(NCG):
        for cc in range(CPG):
            ch = cg * CPG + cc
            for kh in range(3):
                for kw in range(3):
                    for g in range(G):
                        wb[cc * HIN + r + kh, 3 * cg + kw, g * RT + r] = \
                            W9[kh, kw, ch, g] * gscale[g]
    for g in range(G):
        wb[KP, 0, g * RT + r] = gbias[g]
    return xk, wb.astype(ml_dtypes.bfloat16)


def build_program(Tn):
    nc = bacc.Bacc("TRN2", target_bir_lowering=False, debug=False)
    xk_d = nc.dram_tensor("xk", [Tn, KP, NCG, TAU, WP], BF16, kind="ExternalInput")
    wb_d = nc.dram_tensor("wb", [KB, NCG * 3, G * RT], BF16, kind="ExternalInput")
    out_d = nc.dram_tensor("out", [H, W], F32, kind="ExternalOutput")

    segs = sorted(h_window_segments(), key=lambda g: (g[0] % NPAIR, g[0]))

    with tile.TileContext(nc) as tc:
        with tc.tile_pool(name="wpool", bufs=1) as wpool, \
             tc.tile_pool(name="xpool", bufs=3) as xpool, \
             tc.tile_pool(name="gpool", bufs=2) as gpool, \
             tc.tile_pool(name="state", bufs=1) as state, \
             tc.tile_pool(name="zpsum", bufs=2, space="PSUM") as zpsum:

            # --- static weights / state ---
            wt = wpool.tile([KB, NCG * 3, G * RT], BF16, tag="wt", name="wt")
            nc.scalar.dma_start(out=wt, in_=wb_d[:])

            cbuf = state.tile([128, 2, W], BF16, tag="cbuf", name="cbuf")
            nc.vector.memset(cbuf, 0.0)
            hbuf = state.tile([128, 2, WP], BF16, tag="hbuf", name="hbuf")
            nc.vector.memset(hbuf, 0.0)

            # pre-seed the constant-1 bias row in every x-pool buffer
            xbufs = []
            for i in range(3):
                xt = xpool.tile([KB, NCG, TAU, WP], BF16, tag="xt",
                                name=f"xt_pre{i}")
                # engine ops need 32-aligned partition base: memset 96:103
                # with 1.0; the x loads then overwrite 96:102, leaving the
                # constant-1 bias row at partition 102.
                nc.vector.memset(xt[96:KB], 1.0)
                xbufs.append(xt)

            def load_x(t):
                xt = xbufs[t % 3]
                nc.gpsimd.dma_start(out=xt[0:KP], in_=xk_d[t])
                return xt

            xt_cur = load_x(0)
            xt_nxt = load_x(1) if Tn > 1 else None
            for t in range(Tn):
                # --- matmuls: 9 accumulating passes x 4 tau-pairs, N=512 ---
                xv = xt_cur.rearrange("p cg (b q) c -> p cg b q c", b=2)
                zt = [zpsum.tile([G * RT, 2, W], F32, tag=f"z{q}", name=f"z{q}")
                      for q in range(NPAIR)]
                gi = gpool.tile([128, 2, W], BF16, tag="gi", name="gi")
                gf = gpool.tile([128, 2, W], BF16, tag="gf", name="gf")
                go = gpool.tile([128, 2, W], BF16, tag="go", name="go")
                sc = gpool.tile([128, 2, W], BF16, tag="sc", name="sc")
                s2 = gpool.tile([128, 2, W], BF16, tag="s2", name="s2")
                t2 = gpool.tile([128, 2, W], BF16, tag="t2", name="t2")

                for cg in range(2):
                    for kw in range(3):
                        for q in range(NPAIR):
                            nc.tensor.matmul(
                                zt[q], wt[:, 3 * cg + kw],
                                xv[:, cg, :, q, kw:kw + W],
                                start=(cg == 0 and kw == 0), stop=False)

                def emit_windows(xdst, plist):
                    # engine ops can only shift partitions by multiples of
                    # 32; the halo windows need arbitrary shifts -> DMA.
                    engs = (nc.sync, nc.gpsimd)
                    for n, (tau, s, e, p0, blk) in enumerate(plist):
                        engs[n % 2].dma_start(
                            out=xdst[68 + s:68 + e, 2, tau, :],
                            in_=hbuf[p0:p0 + (e - s), blk, :])

                for q in range(NPAIR):
                    for kw in range(3):
                        nc.tensor.matmul(
                            zt[q], wt[:, 6 + kw],
                            xv[:, 2, :, q, kw:kw + W],
                            start=False, stop=(kw == 2))
                    # per-pair PSUM deinterleave (32p forced by PSUM gate
                    # groups); partitions g*32+r -> planar q*32+r, bf16 out.
                    # PE already applied 0.2*z + bias via the constant row.
                    sl = slice(q * RT, (q + 1) * RT)
                    zi, zf, zc, zo = (zt[q][g_ * RT:(g_ + 1) * RT]
                                      for g_ in range(4))
                    nc.scalar.activation(out=sc[sl], in_=zc, func=AF.Sigmoid,
                                         bias=0.0, scale=1.0)
                    nc.scalar.activation(out=go[sl], in_=zo, func=AF.Relu,
                                         bias=0.0, scale=1.0)
                    nc.vector.tensor_scalar(
                        out=gi[sl], in0=zi, scalar1=0.0, scalar2=1.0,
                        op0=OP.max, op1=OP.min)
                    nc.vector.tensor_scalar(
                        out=gf[sl], in0=zf, scalar1=0.0, scalar2=1.0,
                        op0=OP.max, op1=OP.min)

                # --- full-width (128p) bf16 gate math ---
                nc.vector.tensor_scalar(out=go, in0=go, scalar1=1.0,
                                        scalar2=0.0, op0=OP.min, op1=OP.max)
                nc.vector.tensor_tensor(out=t2, in0=gf, in1=cbuf, op=OP.mult)
                nc.vector.tensor_tensor(out=gi, in0=gi, in1=sc, op=OP.mult)
                nc.vector.tensor_tensor(out=cbuf, in0=gi, in1=t2, op=OP.add)
                nc.scalar.activation(out=s2, in_=cbuf, func=AF.Sigmoid,
                                     bias=0.0, scale=1.0)

                if xt_nxt is not None:
                    nc.vector.tensor_tensor(out=hbuf[:, :, 1:W + 1], in0=go,
                                            in1=s2, op=OP.mult)
                    emit_windows(xt_nxt, segs)
                    xt_cur = xt_nxt
                    xt_nxt = load_x(t + 2) if t + 2 < Tn else None
                else:
                    # final step: h = go*s2 per pair -> DRAM
                    hf = gpool.tile([128, 2, W], F32, tag="hf", name="hf")
                    ov = out_d.rearrange("(b p) w -> p b w", p=128)
                    oeng = (nc.sync, nc.gpsimd)
                    for q in range(NPAIR):
                        sl = slice(q * RT, (q + 1) * RT)
                        nc.vector.tensor_tensor(
                            out=hf[sl], in0=go[sl], in1=s2[sl], op=OP.mult)
                        oeng[q % 2].dma_start(out=ov[sl], in_=hf[sl])
    nc.compile()
    return nc


_CACHE = {}


def _get_program(Tn):
    if Tn not in _CACHE:
        _CACHE[Tn] = build_program(Tn)
    return _CACHE[Tn]


def kernel(x, Wx, Wh, b, _run_opts=None):
    x = np.asarray(x, dtype=np.float32)
    Bn, Tn = x.shape[0], x.shape[1]
    xk, wb = pack_inputs(x, Wx, Wh, b)
    nc = _get_program(Tn)
    in_maps = [{"xk": np.ascontiguousarray(xk[bi]), "wb": wb}
               for bi in range(Bn)]
    res = bass_utils.run_bass_kernel_spmd(
        nc, in_maps, core_ids=list(range(Bn)), **(_run_opts or {}))
    out = np.stack([res.results[bi]["out"] for bi in range(Bn)], axis=0)
    kernel.last_results = res
    return out[..., None].astype(np.float32)
